# revision 29
# baseline (speedup 1.0000x reference)
"""Multi-head gated axial attention (width axis) — Trainium2 Bass kernel.

Sharding (8 cores, SPMD, 2 AllToAlls):
  phase 1: pixel-sharded QKV 1x1 conv (core r owns H-rows [8r,8r+8)).
  A2A #1:  core h receives q,k,vT of head h for all pixels.
  phase 2: head-parallel attention; logits^T (w,j) per (n,i) in PSUM;
           per-j rel-bias T2^T (w,ni) in PSUM -> exp -> SBUF;
           E = exp(qk)*exp(T2) (no max subtraction; logits ~|3|);
           x1^T=[vT|1]^T E per i (fused row sums), x2^T per j.
  A2A #2:  per-head unnormalized (x1|sums|x2) back to pixel shards.
  phase 3: y=(x1+x2)*tanh(Gv1)/sums, then wout conv -> out (4096,256).

Scale folding: 1/sqrt(C) in wq; tanh(Gq)/D in rq_t; tanh(Gk)/D in rk_t;
tanh(Gv2)/tanh(Gv1) in rv_t; tanh(Gv1) in the final reciprocal.
"""

import sys

sys.path.insert(0, "/opt/trn_rl_repo")

import numpy as np


def _install_patches():
    import orjson
    import concourse.bass as bass
    import concourse.tile as tile
    from concourse.vector_clock import ScopedClock, VectorClock

    if getattr(bass.Bass, "_wait_split_installed", False):
        return

    def _drain_and_barrier_split(self, tick_clock, wait_clock):
        nc = self.nc
        ticks = list(tick_clock.global_clock)
        for i, t in enumerate(ticks):
            if t <= 0:
                continue
            partial = [0] * len(ticks)
            partial[i] = t
            nop_inst = nc.sync.nop(nofuse=True, hint="drain_split_wait")
            wait_clock.add_sem_waits(
                nop_inst.ins, ScopedClock({None: VectorClock(partial)})
            )
        nc.sync.drain()
        nc.all_engine_barrier()
        assert self.sems is not None
        popped = nc._tile_sem_poison_stack.pop()
        assert popped is self._sem_poison
        nc.clear_and_free_semaphores(list(self.sems.allocated().values()))
        nc.all_engine_barrier()

    tile.TileContext._drain_and_barrier = _drain_and_barrier_split

    counter = [0]

    def _split_waits_json(d):
        for fn in d.get("functions", []):
            for blk in fn.get("blocks", []):
                new_insts = []
                for inst in blk.get("instructions", []):
                    si = inst.get("sync_info") or {}
                    waits = si.get("on_wait") or []
                    if len(waits) > 1:
                        for w in waits[:-1]:
                            counter[0] += 1
                            new_insts.append({
                                "debug": inst.get("debug", 0),
                                "engine": inst["engine"],
                                "ins": [],
                                "name": f"WSPLIT-{counter[0]}",
                                "opcode": "NoOp",
                                "outs": [],
                                "sync_info": {"on_update": [], "on_wait": [w]},
                            })
                        si["on_wait"] = [waits[-1]]
                    new_insts.append(inst)
                blk["instructions"] = new_insts
        return d

    orig_tjb = bass.Bass.to_json_bytes

    def to_json_bytes(self):
        return orjson.dumps(_split_waits_json(orjson.loads(orig_tjb(self))))

    bass.Bass.to_json_bytes = to_json_bytes
    bass.Bass._wait_split_installed = True


_install_patches()

import concourse.bass as bass
import concourse.mybir as mybir
import concourse.tile as tile
from concourse.bass_utils import run_bass_kernel_spmd
from concourse.masks import make_identity

N, C, H, W = 4, 256, 64, 128
NH, HD = 8, 32
D = float(np.sqrt(C))
NCORES = 8
HSH = H // NCORES
PIX = N * HSH * W      # 4096 local pixels, order (n, i_local, w)
NI = N * H             # 256 (n,i) rows; i-enum = (s, n, il) -> rank-s pixels
FP32 = mybir.dt.float32
BF16 = mybir.dt.bfloat16
SH1 = HD * PIX * 3     # A2A1 shard: [q (32,PIX) | k (32,PIX) | vT (PIX,32)]
SH2 = 65 * PIX         # A2A2 shard: [x1+sums (33,PIX) | x2 (32,PIX)]
Tanh = mybir.ActivationFunctionType.Tanh
Exp = mybir.ActivationFunctionType.Exp


DEBUG = False


def _build():
    nc = bass.Bass()
    x_s = nc.declare_dram_parameter("x_s", [C, PIX], FP32, isOutput=False)
    wqkvT = nc.declare_dram_parameter("wqkvT", [C, 3 * C], FP32, isOutput=False)
    woutT = nc.declare_dram_parameter("woutT", [C, C], FP32, isOutput=False)
    rq_h = nc.declare_dram_parameter("rq_h", [HD, W, W], FP32, isOutput=False)
    rk_h = nc.declare_dram_parameter("rk_h", [HD, W, W], FP32, isOutput=False)
    rv_h = nc.declare_dram_parameter("rv_h", [HD, W, W], FP32, isOutput=False)
    g2 = nc.declare_dram_parameter("g2", [4], FP32, isOutput=False)
    gv1a = nc.declare_dram_parameter("gv1a", [NH], FP32, isOutput=False)
    gv2a = nc.declare_dram_parameter("gv2a", [NH], FP32, isOutput=False)
    out = nc.declare_dram_parameter("out", [PIX, C], FP32, isOutput=True)

    def drain(i, dst, src):
        if i % 2:
            nc.scalar.copy(dst, src)
        else:
            nc.vector.tensor_copy(dst, src)

    with tile.TileContext(nc) as tc:
        with tc.tile_pool(name="dram", bufs=1, space="DRAM") as dram, \
             tc.tile_pool(name="persist", bufs=1) as per, \
             tc.tile_pool(name="psum", bufs=6, space="PSUM") as psp:

            mine1 = dram.tile([NCORES * SH1], BF16)
            gath1 = dram.tile([NCORES * SH1], BF16)
            mine2 = dram.tile([NCORES * SH2], BF16)
            gath2 = dram.tile([NCORES * SH2], BF16)

            # =============== phase 1: qkv conv on pixel shard ===============
            with tc.tile_pool(name="ph1", bufs=1) as p1, \
                 tc.tile_pool(name="st1", bufs=2) as st1:
                xf = p1.tile([128, 2, PIX], FP32)
                nc.gpsimd.dma_start(
                    xf[:], x_s.rearrange("(kt p) f -> p kt f", p=128))
                xb = p1.tile([128, 2, PIX], BF16)
                nc.vector.tensor_copy(xb[:], xf[:])
                wf = p1.tile([128, 2, 3 * C], FP32)
                nc.gpsimd.dma_start(
                    wf[:], wqkvT.rearrange("(kt p) f -> p kt f", p=128))
                nc.scalar.mul(wf[:, :, 0:C], wf[:, :, 0:C], 1.0 / D)
                wb = p1.tile([128, 2, 3 * C], BF16)
                nc.vector.tensor_copy(wb[:], wf[:])

                # q,k: out (oc,pix); shard j holds q_j (c,f) then k_j (c,f)
                m1qk = mine1.rearrange(
                    "(j t c f) -> j t c f", j=NH, t=3, c=HD)
                for m in range(4):
                    t, j0 = m // 2, 4 * (m % 2)
                    for pc in range(8):
                        ps = psp.tile([128, 512], FP32, tag="ps")
                        for kt in range(2):
                            nc.tensor.matmul(
                                ps[:], wb[:, kt, m * 128:(m + 1) * 128],
                                xb[:, kt, pc * 512:(pc + 1) * 512],
                                start=(kt == 0), stop=(kt == 1))
                        sb = st1.tile([128, 512], BF16, tag="qks")
                        drain(m + pc, sb[:], ps[:])
                        for jj in range(4):
                            nc.sync.dma_start(
                                m1qk[j0 + jj, t, :, pc * 512:(pc + 1) * 512],
                                sb[jj * 32:(jj + 1) * 32, :])
                # v: out (pix, vc); shard j block t=2, flat lp*HD + c
                m1v = mine1.rearrange(
                    "(j t lp c) -> j t lp c", j=NH, t=3, lp=PIX)[:, 2] \
                    .rearrange("j lp c -> lp j c")
                for pt in range(32):
                    ps2 = psp.tile([128, C], FP32, tag="ps")
                    for kt in range(2):
                        nc.tensor.matmul(
                            ps2[:], xb[:, kt, pt * 128:(pt + 1) * 128],
                            wb[:, kt, 2 * C:3 * C],
                            start=(kt == 0), stop=(kt == 1))
                    sb2 = st1.tile([128, C], BF16, tag="vs")
                    drain(pt, sb2[:], ps2[:])
                    nc.sync.dma_start(
                        m1v[pt * 128:(pt + 1) * 128],
                        sb2[:].rearrange("p (j c) -> p j c", j=NH))

            nc.gpsimd.collective_compute(
                "AllToAll", mybir.AluOpType.bypass,
                replica_groups=[list(range(NCORES))],
                ins=[mine1.opt()], outs=[gath1.opt()])

            # ====== prep (overlaps A2A1): gates + rv tanh/transpose ======
            g1qk = gath1.rearrange("(s t c f) -> s t c f", s=NH, t=3, c=HD)
            with tc.tile_pool(name="rpool2", bufs=1) as rp2:
              rvT_t = rp2.tile([W, W, HD], BF16)
              E_s = per.tile([W, W, NI], BF16)   # (w, j, ni)
              with tc.tile_pool(name="prep", bufs=2) as pp:
                    gv_t = pp.tile([HD, 1], FP32, tag="g3")
                    nc.gpsimd.dma_start(gv_t[:], g2[3:4].to_broadcast((HD, 1)))
                    nc.scalar.activation(gv_t[:], gv_t[:], Tanh)
                    gv1m = pp.tile([HD, 1], FP32, tag="g4")
                    nc.gpsimd.dma_start(gv1m[:], g2[2:3].to_broadcast((HD, 1)))
                    nc.scalar.activation(gv1m[:], gv1m[:], Tanh)
                    rcp1 = pp.tile([HD, 1], FP32, tag="g5")
                    nc.vector.reciprocal(rcp1[:], gv1m[:])
                    nc.vector.tensor_mul(gv_t[:], gv_t[:], rcp1[:])
                    ident = pp.tile([HD, HD], BF16, tag="id")
                    make_identity(nc, ident[:])
                    for ch in range(4):
                        rst = pp.tile([HD, 32, W], FP32, tag="rst")
                        nc.gpsimd.dma_start(
                            rst[:], rv_h[:, ch * 32:(ch + 1) * 32, :])
                        rvb = pp.tile([HD, 32, W], BF16, tag="rvb")
                        nc.scalar.activation(rvb[:], rst[:], Tanh)
                        nc.vector.tensor_scalar_mul(rvb[:], rvb[:], gv_t[:])
                        for jl in range(32):
                            pst = psp.tile([W, HD], BF16, tag="ps")
                            nc.tensor.transpose(pst[:], rvb[:, jl, :], ident[:])
                            drain(jl, rvT_t[:, ch * 32 + jl, :], pst[:])

              # ===== pass A: E_s = exp(qrq), q resident =====
              with tc.tile_pool(name="p2a", bufs=1) as p2a, \
                   tc.tile_pool(name="p2as", bufs=2) as p2as:
                    gq_t = p2a.tile([HD, 1], FP32, tag="g1")
                    nc.gpsimd.dma_start(gq_t[:], g2[0:1].to_broadcast((HD, 1)))
                    nc.scalar.activation(gq_t[:], gq_t[:], Tanh)
                    nc.scalar.mul(gq_t[:], gq_t[:], 1.0 / D)
                    rq_t = p2a.tile([HD, W, W], BF16)
                    for ch in range(8):
                        rst = p2as.tile([HD, 16, W], FP32, tag="rst")
                        nc.gpsimd.dma_start(
                            rst[:], rq_h[:, ch * 16:(ch + 1) * 16, :])
                        nc.scalar.activation(
                            rq_t[:, ch * 16:(ch + 1) * 16, :], rst[:], Tanh)
                        nc.vector.tensor_scalar_mul(
                            rq_t[:, ch * 16:(ch + 1) * 16, :],
                            rq_t[:, ch * 16:(ch + 1) * 16, :], gq_t[:])
                    q_h = p2a.tile([HD, NCORES, PIX], BF16)
                    for s in range(NCORES):
                        nc.sync.dma_start(q_h[:, s], g1qk[s, 0])
                    q_j = q_h.rearrange("c s (nl j) -> c j (s nl)", j=W)
                    for j in range(W):
                        psQ = psp.tile([W, NI], FP32, tag="ps")
                        nc.tensor.matmul(psQ[:], rq_t[:, j, :], q_j[:, j, :],
                                         start=True, stop=True)
                        nc.scalar.activation(E_s[:, j, :], psQ[:], Exp)

              # ===== pass B: E_s *= exp(krk); then per-i qk-mult + x1 =====
              m2v = mine2.rearrange("(r t i2 j) -> r t i2 j",
                                    r=NCORES, t=65, j=W)
              with tc.tile_pool(name="p2c", bufs=1) as p2c, \
                   tc.tile_pool(name="p2b", bufs=1) as p2b:
                  vTo = p2c.tile([W, NI, HD + 1], BF16)
                  nc.vector.memset(vTo[:, :, HD:HD + 1], 1.0)
                  g1v = gath1.rearrange("(s t nl w c) -> s t nl w c",
                                        s=NH, t=3, nl=PIX // W, w=W)
                  for s in range(NCORES):
                      nc.sync.dma_start(
                          vTo[:, s * (PIX // W):(s + 1) * (PIX // W), 0:HD],
                          g1v[s, 2].rearrange("nl w c -> w nl c"))

                  k_h = p2b.tile([HD, NCORES, PIX], BF16)
                  for s in range(NCORES):
                      nc.sync.dma_start(k_h[:, s], g1qk[s, 1])
                  with tc.tile_pool(name="rkp", bufs=1) as rkp, \
                       tc.tile_pool(name="rks", bufs=2) as rks:
                    gk_t = rkp.tile([HD, 1], FP32, tag="g2")
                    nc.gpsimd.dma_start(gk_t[:], g2[1:2].to_broadcast((HD, 1)))
                    nc.scalar.activation(gk_t[:], gk_t[:], Tanh)
                    nc.scalar.mul(gk_t[:], gk_t[:], 1.0 / D)
                    rk_t = rkp.tile([HD, W, W], BF16)
                    for ch in range(8):
                        rst = rks.tile([HD, 16, W], FP32, tag="rst")
                        nc.gpsimd.dma_start(
                            rst[:], rk_h[:, ch * 16:(ch + 1) * 16, :])
                        nc.scalar.activation(
                            rk_t[:, ch * 16:(ch + 1) * 16, :], rst[:], Tanh)
                        nc.vector.tensor_scalar_mul(
                            rk_t[:, ch * 16:(ch + 1) * 16, :],
                            rk_t[:, ch * 16:(ch + 1) * 16, :], gk_t[:])
                    k_j = k_h.rearrange("c s (nl j) -> c j (s nl)", j=W)
                    for j in range(W):
                        psK = psp.tile([W, NI], FP32, tag="ps")
                        nc.tensor.matmul(psK[:], rk_t[:, j, :], k_j[:, j, :],
                                         start=True, stop=True)
                        et = rks.tile([W, NI], BF16, tag="et")
                        nc.scalar.activation(et[:], psK[:], Exp)
                        nc.vector.tensor_mul(E_s[:, j, :], E_s[:, j, :], et[:])
                  with tc.tile_pool(name="p2bs", bufs=3) as p2bs:
                    k_i = k_h.rearrange("c s f -> c (s f)")
                    for sc in range(NCORES):
                        qc = p2bs.tile([HD, PIX], BF16, tag="qc")
                        nc.sync.dma_start(qc[:], g1qk[sc, 0])
                        x1blk = p2bs.tile([HD + 1, 32, W], BF16, tag="x1b")
                        for il in range(PIX // W):
                            i = sc * (PIX // W) + il
                            psL = psp.tile([W, W], FP32, tag="ps")
                            nc.tensor.matmul(
                                psL[:], k_i[:, i * W:(i + 1) * W],
                                qc[:, il * W:(il + 1) * W],
                                start=True, stop=True)
                            eq = p2bs.tile([W, W], BF16, tag="eq")
                            nc.scalar.activation(eq[:], psL[:], Exp)
                            nc.vector.tensor_mul(
                                E_s[:, :, i], E_s[:, :, i], eq[:])
                            psX = psp.tile([HD + 1, W], FP32, tag="ps")
                            nc.tensor.matmul(
                                psX[:], vTo[:, i, :], E_s[:, :, i],
                                start=True, stop=True)
                            drain(i, x1blk[:, il, :], psX[:])
                        nc.sync.dma_start(m2v[sc, 0:33], x1blk[:])

              # ===== phase 2b: x2 =====
              with tc.tile_pool(name="p2d", bufs=1) as p2c:
                x2st4 = p2c.tile([128, NI, 32], BF16)
                E_j = E_s
                for g in range(32):
                    ps4 = psp.tile([128, NI], FP32, tag="ps")
                    for a in range(4):
                        j = a * 32 + g
                        nc.tensor.matmul(
                            ps4[a * 32:(a + 1) * 32, :],
                            rvT_t[:, j, :], E_j[:, j, :],
                            start=True, stop=True,
                            tile_position=(0, a * 32))
                    drain(g, x2st4[:, :, g], ps4[:])
                for r in range(NCORES):
                    for a in range(4):
                        nc.sync.dma_start(
                            m2v[r, 33:65, :, a * 32:(a + 1) * 32],
                            x2st4[a * 32:(a + 1) * 32,
                                  r * 32:(r + 1) * 32, :])

            nc.gpsimd.collective_compute(
                "AllToAll", mybir.AluOpType.bypass,
                replica_groups=[list(range(NCORES))],
                ins=[mine2.opt()], outs=[gath2.opt()])

            # =============== phase 3: merge + normalize + wout conv =========
            with tc.tile_pool(name="ph3", bufs=1) as p3, \
                 tc.tile_pool(name="st3", bufs=2) as st3:
                g2v = gath2.rearrange("(hh t f) -> hh t f", hh=NH, t=65)
                y1b = p3.tile([128, 2, PIX], BF16)
                x2b = p3.tile([128, 2, PIX], BF16)
                sums = p3.tile([NH, PIX], BF16)
                for hh in range(NH):
                    kt, po = divmod(hh, 4)
                    nc.sync.dma_start(
                        y1b[po * 32:(po + 1) * 32, kt, :], g2v[hh, 0:HD])
                    nc.sync.dma_start(
                        x2b[po * 32:(po + 1) * 32, kt, :],
                        g2v[hh, 33:33 + HD])
                    nc.sync.dma_start(
                        sums[hh:hh + 1, :], g2v[hh, HD:HD + 1])
                nc.vector.tensor_add(y1b[:], y1b[:], x2b[:])
                gv1_8 = p3.tile([NH, 1], FP32)
                nc.gpsimd.dma_start(
                    gv1_8[:], gv1a.rearrange("(h o) -> h o", o=1))
                nc.scalar.activation(gv1_8[:], gv1_8[:], Tanh)
                rsr = p3.tile([NH, PIX], FP32)
                nc.vector.reciprocal(rsr[:], sums[:])
                nc.vector.tensor_scalar_mul(rsr[:], rsr[:], gv1_8[:])
                rsd = dram.tile([NH, PIX], FP32)
                nc.sync.dma_start(rsd[:], rsr[:])
                rsx = p3.tile([128, 2, PIX], FP32)
                for hh in range(NH):
                    kt, po = divmod(hh, 4)
                    nc.sync.dma_start(
                        rsx[po * 32:(po + 1) * 32, kt, :],
                        rsd[hh:hh + 1, :].to_broadcast((32, PIX)))
                yb = p3.tile([128, 2, PIX], BF16)
                nc.vector.tensor_mul(yb[:], y1b[:], rsx[:])

                wof = p3.tile([128, 2, C], FP32)
                nc.gpsimd.dma_start(
                    wof[:], woutT.rearrange("(kt p) f -> p kt f", p=128))
                wob = p3.tile([128, 2, C], BF16)
                nc.vector.tensor_copy(wob[:], wof[:])
                if DEBUG:
                    with tc.tile_pool(name="dbg", bufs=1) as dbp:
                        ovf = out.rearrange("(pix) c -> (pix c)")
                        def dump(dst_off, src_ap, shape):
                            P0, F0 = shape
                            for f0 in range(0, F0, 1024):
                                fl = min(1024, F0 - f0)
                                t_b = dbp.tile([P0, fl], BF16, tag="db")
                                nc.sync.dma_start(t_b[:], src_ap[:, f0:f0 + fl])
                                t_f = dbp.tile([P0, fl], FP32, tag="df")
                                nc.vector.tensor_copy(t_f[:], t_b[:])
                                nc.sync.dma_start(
                                    ovf[dst_off + f0 * P0:
                                        dst_off + (f0 + fl) * P0].rearrange(
                                        "(p f) -> p f", p=P0), t_f[:])
                        g1f = gath1.rearrange("(a) -> a")
                        dump(0, g1f[0:HD * PIX].rearrange("(c f) -> c f", c=HD),
                             (HD, PIX))                      # q of my head, rank0 pixels
                        g2f = gath2.rearrange("(a) -> a")
                        dump(HD * PIX, g2f[0:33 * PIX].rearrange(
                            "(t f) -> t f", t=33), (33, PIX))  # head0 x1+sums, my pixels
                        e0 = dbp.tile([W, W], FP32, tag="e0")
                        nc.vector.tensor_copy(e0[:], E_s[:, :, 0].rearrange('w j -> w j'))
                        nc.sync.dma_start(
                            ovf[300000:300000 + W * W].rearrange(
                                "(p f) -> p f", p=W), e0[:])   # E_s[:, 0, :]
                        dump(400000, g1f[HD * PIX:2 * HD * PIX].rearrange(
                            "(c f) -> c f", c=HD), (HD, PIX))   # k head, rank0
                if not DEBUG:
                    ov = out.rearrange("(pt p) c -> p pt c", p=128)
                    for pt in range(32):
                        psO = psp.tile([128, C], FP32, tag="ps")
                        for kt in range(2):
                            nc.tensor.matmul(
                                psO[:], yb[:, kt, pt * 128:(pt + 1) * 128],
                                wob[:, kt, :], start=(kt == 0), stop=(kt == 1))
                        so = st3.tile([128, C], FP32, tag="os")
                        drain(pt, so[:], psO[:])
                        nc.sync.dma_start(ov[:, pt, :], so[:])

    nc.finalize()
    return nc


_BUILT = None


def kernel(x, wq, wk, wv, wout, rq, rk, rv, Gq, Gk, Gv1, Gv2):
    global _BUILT
    x = np.asarray(x, np.float32)
    wqkvT = np.concatenate(
        [np.asarray(wq, np.float32).T, np.asarray(wk, np.float32).T,
         np.asarray(wv, np.float32).T], axis=1).copy()
    woutT = np.ascontiguousarray(np.asarray(wout, np.float32).T)
    rq = np.asarray(rq, np.float32)
    rk = np.asarray(rk, np.float32)
    rv = np.asarray(rv, np.float32)
    Gq = np.asarray(Gq, np.float32)
    Gk = np.asarray(Gk, np.float32)
    Gv1 = np.asarray(Gv1, np.float32)
    Gv2 = np.asarray(Gv2, np.float32)

    if _BUILT is None:
        _BUILT = _build()
    nc = _BUILT

    in_maps = []
    for r in range(NCORES):
        xs = np.ascontiguousarray(
            x[:, :, r * HSH:(r + 1) * HSH, :].transpose(1, 0, 2, 3)
        ).reshape(C, PIX)
        in_maps.append({
            "x_s": xs,
            "wqkvT": wqkvT,
            "woutT": woutT,
            "rq_h": np.ascontiguousarray(rq[r]),
            "rk_h": np.ascontiguousarray(rk[r]),
            "rv_h": np.ascontiguousarray(rv[r]),
            "g2": np.array([Gq[r], Gk[r], Gv1[r], Gv2[r]], np.float32),
            "gv1a": Gv1.copy(),
            "gv2a": Gv2.copy(),
        })

    res = run_bass_kernel_spmd(nc, in_maps, core_ids=list(range(NCORES)))
    global _LAST_RESULT
    _LAST_RESULT = res
    outf = np.empty((N, C, H, W), np.float32)
    for r in range(NCORES):
        o = res.results[r]["out"].reshape(N, HSH, W, C).transpose(0, 3, 1, 2)
        outf[:, :, r * HSH:(r + 1) * HSH, :] = o
    return outf


# revision 33
# speedup vs baseline: 1.2974x; 1.2974x over previous
"""Multi-head gated axial attention (width axis) — Trainium2 Bass kernel.

Sharding (8 cores, SPMD, 2 AllToAlls):
  phase 1: pixel-sharded QKV 1x1 conv (core r owns H-rows [8r,8r+8)).
  A2A #1:  core h receives q,k,vT of head h for all pixels.
  phase 2: head-parallel attention; logits^T (w,j) per (n,i) in PSUM;
           per-j rel-bias T2^T (w,ni) in PSUM -> exp -> SBUF;
           E = exp(qk)*exp(T2) (no max subtraction; logits ~|3|);
           x1^T=[vT|1]^T E per i (fused row sums), x2^T per j.
  A2A #2:  per-head unnormalized (x1|sums|x2) back to pixel shards.
  phase 3: y=(x1+x2)*tanh(Gv1)/sums, then wout conv -> out (4096,256).

Scale folding: 1/sqrt(C) in wq; tanh(Gq)/D in rq_t; tanh(Gk)/D in rk_t;
tanh(Gv2)/tanh(Gv1) in rv_t; tanh(Gv1) in the final reciprocal.
"""

import sys

sys.path.insert(0, "/opt/trn_rl_repo")

import numpy as np


def _install_patches():
    import orjson
    import concourse.bass as bass
    import concourse.tile as tile
    from concourse.vector_clock import ScopedClock, VectorClock

    if getattr(bass.Bass, "_wait_split_installed", False):
        return

    def _drain_and_barrier_split(self, tick_clock, wait_clock):
        nc = self.nc
        ticks = list(tick_clock.global_clock)
        for i, t in enumerate(ticks):
            if t <= 0:
                continue
            partial = [0] * len(ticks)
            partial[i] = t
            nop_inst = nc.sync.nop(nofuse=True, hint="drain_split_wait")
            wait_clock.add_sem_waits(
                nop_inst.ins, ScopedClock({None: VectorClock(partial)})
            )
        nc.sync.drain()
        nc.all_engine_barrier()
        assert self.sems is not None
        popped = nc._tile_sem_poison_stack.pop()
        assert popped is self._sem_poison
        nc.clear_and_free_semaphores(list(self.sems.allocated().values()))
        nc.all_engine_barrier()

    tile.TileContext._drain_and_barrier = _drain_and_barrier_split

    counter = [0]

    def _split_waits_json(d):
        for fn in d.get("functions", []):
            for blk in fn.get("blocks", []):
                new_insts = []
                for inst in blk.get("instructions", []):
                    si = inst.get("sync_info") or {}
                    waits = si.get("on_wait") or []
                    if len(waits) > 1:
                        for w in waits[:-1]:
                            counter[0] += 1
                            new_insts.append({
                                "debug": inst.get("debug", 0),
                                "engine": inst["engine"],
                                "ins": [],
                                "name": f"WSPLIT-{counter[0]}",
                                "opcode": "NoOp",
                                "outs": [],
                                "sync_info": {"on_update": [], "on_wait": [w]},
                            })
                        si["on_wait"] = [waits[-1]]
                    new_insts.append(inst)
                blk["instructions"] = new_insts
        return d

    orig_tjb = bass.Bass.to_json_bytes

    def to_json_bytes(self):
        return orjson.dumps(_split_waits_json(orjson.loads(orig_tjb(self))))

    bass.Bass.to_json_bytes = to_json_bytes
    bass.Bass._wait_split_installed = True


_install_patches()

import concourse.bass as bass
import concourse.mybir as mybir
import concourse.tile as tile
from concourse.bass_utils import run_bass_kernel_spmd
from concourse.masks import make_identity

N, C, H, W = 4, 256, 64, 128
NH, HD = 8, 32
D = float(np.sqrt(C))
NCORES = 8
HSH = H // NCORES
PIX = N * HSH * W      # 4096 local pixels, order (n, i_local, w)
NI = N * H             # 256 (n,i) rows; i-enum = (s, n, il) -> rank-s pixels
FP32 = mybir.dt.float32
BF16 = mybir.dt.bfloat16
SH1 = HD * PIX * 3     # A2A1 shard: [q (32,PIX) | k (32,PIX) | vT (PIX,32)]
SH2 = 65 * PIX         # A2A2 shard: [x1+sums (33,PIX) | x2 (32,PIX)]
Tanh = mybir.ActivationFunctionType.Tanh
Exp = mybir.ActivationFunctionType.Exp


DEBUG = False


def _build():
    nc = bass.Bass()
    x_s = nc.declare_dram_parameter("x_s", [C, PIX], FP32, isOutput=False)
    wqkvT = nc.declare_dram_parameter("wqkvT", [C, 3 * C], FP32, isOutput=False)
    woutT = nc.declare_dram_parameter("woutT", [C, C], FP32, isOutput=False)
    rq_h = nc.declare_dram_parameter("rq_h", [HD, W, W], FP32, isOutput=False)
    rk_h = nc.declare_dram_parameter("rk_h", [HD, W, W], FP32, isOutput=False)
    rv_h = nc.declare_dram_parameter("rv_h", [HD, W, W], FP32, isOutput=False)
    g2 = nc.declare_dram_parameter("g2", [4], FP32, isOutput=False)
    gv1a = nc.declare_dram_parameter("gv1a", [NH], FP32, isOutput=False)
    gv2a = nc.declare_dram_parameter("gv2a", [NH], FP32, isOutput=False)
    out = nc.declare_dram_parameter("out", [PIX, C], FP32, isOutput=True)

    def drain(i, dst, src):
        if i % 2:
            nc.scalar.copy(dst, src)
        else:
            nc.vector.tensor_copy(dst, src)

    with tile.TileContext(nc) as tc:
        with tc.tile_pool(name="dram", bufs=1, space="DRAM") as dram, \
             tc.tile_pool(name="persist", bufs=1) as per, \
             tc.tile_pool(name="psum", bufs=6, space="PSUM") as psp:

            mine1 = dram.tile([NCORES * SH1], BF16)
            gath1 = dram.tile([NCORES * SH1], BF16)
            mine2 = dram.tile([NCORES * SH2], BF16)
            gath2 = dram.tile([NCORES * SH2], BF16)

            # =============== phase 1: qkv conv on pixel shard ===============
            with tc.tile_pool(name="ph1", bufs=1) as p1, \
                 tc.tile_pool(name="st1", bufs=2) as st1:
                xf = p1.tile([128, 2, PIX], FP32)
                nc.gpsimd.dma_start(
                    xf[:], x_s.rearrange("(kt p) f -> p kt f", p=128))
                xb = p1.tile([128, 2, PIX], BF16)
                nc.vector.tensor_copy(xb[:], xf[:])
                wf = p1.tile([128, 2, 3 * C], FP32)
                nc.gpsimd.dma_start(
                    wf[:], wqkvT.rearrange("(kt p) f -> p kt f", p=128))
                nc.scalar.mul(wf[:, :, 0:C], wf[:, :, 0:C], 1.0 / D)
                wb = p1.tile([128, 2, 3 * C], BF16)
                nc.vector.tensor_copy(wb[:], wf[:])

                # q,k: out (oc,pix); shard j holds q_j (c,f) then k_j (c,f)
                m1qk = mine1.rearrange(
                    "(j t c f) -> j t c f", j=NH, t=3, c=HD)
                for m in range(4):
                    t, j0 = m // 2, 4 * (m % 2)
                    for pc in range(8):
                        ps = psp.tile([128, 512], FP32, tag="ps")
                        for kt in range(2):
                            nc.tensor.matmul(
                                ps[:], wb[:, kt, m * 128:(m + 1) * 128],
                                xb[:, kt, pc * 512:(pc + 1) * 512],
                                start=(kt == 0), stop=(kt == 1))
                        sb = st1.tile([128, 512], BF16, tag="qks")
                        drain(m + pc, sb[:], ps[:])
                        for jj in range(4):
                            nc.sync.dma_start(
                                m1qk[j0 + jj, t, :, pc * 512:(pc + 1) * 512],
                                sb[jj * 32:(jj + 1) * 32, :])
                # v: out (pix, vc); shard j block t=2, flat lp*HD + c
                m1v = mine1.rearrange(
                    "(j t lp c) -> j t lp c", j=NH, t=3, lp=PIX)[:, 2] \
                    .rearrange("j lp c -> lp j c")
                for pt in range(32):
                    ps2 = psp.tile([128, C], FP32, tag="ps")
                    for kt in range(2):
                        nc.tensor.matmul(
                            ps2[:], xb[:, kt, pt * 128:(pt + 1) * 128],
                            wb[:, kt, 2 * C:3 * C],
                            start=(kt == 0), stop=(kt == 1))
                    sb2 = st1.tile([128, C], BF16, tag="vs")
                    drain(pt, sb2[:], ps2[:])
                    nc.sync.dma_start(
                        m1v[pt * 128:(pt + 1) * 128],
                        sb2[:].rearrange("p (j c) -> p j c", j=NH))

            nc.gpsimd.collective_compute(
                "AllToAll", mybir.AluOpType.bypass,
                replica_groups=[list(range(NCORES))],
                ins=[mine1.opt()], outs=[gath1.opt()])

            # ====== prep (overlaps A2A1): gates + rv tanh/transpose ======
            g1qk = gath1.rearrange("(s t c f) -> s t c f", s=NH, t=3, c=HD)
            with tc.tile_pool(name="rpool2", bufs=1) as rp2:
              rvT_t = rp2.tile([W, W, HD], BF16)
              E_s = per.tile([W, W, NI], BF16)   # (w, j, ni)
              with tc.tile_pool(name="prep", bufs=2) as pp:
                    gv_t = pp.tile([HD, 1], FP32, tag="g3")
                    nc.gpsimd.dma_start(gv_t[:], g2[3:4].to_broadcast((HD, 1)))
                    nc.scalar.activation(gv_t[:], gv_t[:], Tanh)
                    gv1m = pp.tile([HD, 1], FP32, tag="g4")
                    nc.gpsimd.dma_start(gv1m[:], g2[2:3].to_broadcast((HD, 1)))
                    nc.scalar.activation(gv1m[:], gv1m[:], Tanh)
                    rcp1 = pp.tile([HD, 1], FP32, tag="g5")
                    nc.vector.reciprocal(rcp1[:], gv1m[:])
                    nc.vector.tensor_mul(gv_t[:], gv_t[:], rcp1[:])
                    ident = pp.tile([HD, HD], BF16, tag="id")
                    make_identity(nc, ident[:])
                    for ch in range(4):
                        rst = pp.tile([HD, 32, W], FP32, tag="rst")
                        nc.gpsimd.dma_start(
                            rst[:], rv_h[:, ch * 32:(ch + 1) * 32, :])
                        rvb = pp.tile([HD, 32, W], BF16, tag="rvb")
                        nc.scalar.activation(rvb[:], rst[:], Tanh)
                        nc.vector.tensor_scalar_mul(rvb[:], rvb[:], gv_t[:])
                        for jl in range(32):
                            pst = psp.tile([W, HD], BF16, tag="ps")
                            nc.tensor.transpose(pst[:], rvb[:, jl, :], ident[:])
                            drain(jl, rvT_t[:, ch * 32 + jl, :], pst[:])

              # ===== pass A: E_s = exp(qrq), q resident =====
              with tc.tile_pool(name="p2a", bufs=1) as p2a, \
                   tc.tile_pool(name="p2as", bufs=2) as p2as:
                    gq_t = p2a.tile([HD, 1], FP32, tag="g1")
                    nc.gpsimd.dma_start(gq_t[:], g2[0:1].to_broadcast((HD, 1)))
                    nc.scalar.activation(gq_t[:], gq_t[:], Tanh)
                    nc.scalar.mul(gq_t[:], gq_t[:], 1.0 / D)
                    rq_t = p2a.tile([HD, W, W], BF16)
                    for ch in range(8):
                        rst = p2as.tile([HD, 16, W], FP32, tag="rst")
                        nc.gpsimd.dma_start(
                            rst[:], rq_h[:, ch * 16:(ch + 1) * 16, :])
                        nc.scalar.activation(
                            rq_t[:, ch * 16:(ch + 1) * 16, :], rst[:], Tanh)
                        nc.vector.tensor_scalar_mul(
                            rq_t[:, ch * 16:(ch + 1) * 16, :],
                            rq_t[:, ch * 16:(ch + 1) * 16, :], gq_t[:])
                    q_h = p2a.tile([HD, NCORES, PIX], BF16)
                    for s in range(NCORES):
                        nc.sync.dma_start(q_h[:, s], g1qk[s, 0])
                    q_j = q_h.rearrange("c s (nl j) -> c j (s nl)", j=W)
                    for j in range(W):
                        psQ = psp.tile([W, NI], FP32, tag="ps")
                        nc.tensor.matmul(psQ[:], rq_t[:, j, :], q_j[:, j, :],
                                         start=True, stop=True)
                        nc.scalar.activation(E_s[:, j, :], psQ[:], Exp)

              # ===== pass B: E_s *= exp(krk); then per-i qk-mult + x1 =====
              m2v = mine2.rearrange("(r t i2 j) -> r t i2 j",
                                    r=NCORES, t=65, j=W)
              with tc.tile_pool(name="p2b", bufs=1) as p2b:

                  k_h = p2b.tile([HD, NCORES, PIX], BF16)
                  for s in range(NCORES):
                      nc.sync.dma_start(k_h[:, s], g1qk[s, 1])
                  with tc.tile_pool(name="rkp", bufs=1) as rkp, \
                       tc.tile_pool(name="rks", bufs=2) as rks:
                    gk_t = rkp.tile([HD, 1], FP32, tag="g2")
                    nc.gpsimd.dma_start(gk_t[:], g2[1:2].to_broadcast((HD, 1)))
                    nc.scalar.activation(gk_t[:], gk_t[:], Tanh)
                    nc.scalar.mul(gk_t[:], gk_t[:], 1.0 / D)
                    rk_t = rkp.tile([HD, W, W], BF16)
                    for ch in range(8):
                        rst = rks.tile([HD, 16, W], FP32, tag="rst")
                        nc.gpsimd.dma_start(
                            rst[:], rk_h[:, ch * 16:(ch + 1) * 16, :])
                        nc.scalar.activation(
                            rk_t[:, ch * 16:(ch + 1) * 16, :], rst[:], Tanh)
                        nc.vector.tensor_scalar_mul(
                            rk_t[:, ch * 16:(ch + 1) * 16, :],
                            rk_t[:, ch * 16:(ch + 1) * 16, :], gk_t[:])
                    k_j = k_h.rearrange("c s (nl j) -> c j (s nl)", j=W)
                    for j in range(W):
                        psK = psp.tile([W, NI], FP32, tag="ps")
                        nc.tensor.matmul(psK[:], rk_t[:, j, :], k_j[:, j, :],
                                         start=True, stop=True)
                        et = rks.tile([W, NI], BF16, tag="et")
                        nc.scalar.activation(et[:], psK[:], Exp)
                        nc.vector.tensor_mul(E_s[:, j, :], E_s[:, j, :], et[:])
                  with tc.tile_pool(name="p2bs", bufs=3) as p2bs:
                    k_i = k_h.rearrange("c s f -> c (s f)")
                    for sc in range(NCORES):
                        qc = p2bs.tile([HD, PIX], BF16, tag="qc")
                        nc.sync.dma_start(qc[:], g1qk[sc, 0])
                        for il in range(PIX // W):
                            i = sc * (PIX // W) + il
                            psL = psp.tile([W, W], FP32, tag="ps")
                            nc.tensor.matmul(
                                psL[:], k_i[:, i * W:(i + 1) * W],
                                qc[:, il * W:(il + 1) * W],
                                start=True, stop=True)
                            eq = p2bs.tile([W, W], BF16, tag="eq")
                            nc.scalar.activation(eq[:], psL[:], Exp)
                            nc.vector.tensor_mul(
                                E_s[:, :, i], E_s[:, :, i], eq[:])

              # ===== phase 2b: x1 then x2 =====
              with tc.tile_pool(name="p2d", bufs=1) as p2c, \
                   tc.tile_pool(name="p2ds", bufs=3) as p2ds:
                vTo = p2c.tile([W, NI, HD + 1], BF16)
                nc.vector.memset(vTo[:, :, HD:HD + 1], 1.0)
                g1v = gath1.rearrange("(s t nl w c) -> s t nl w c",
                                      s=NH, t=3, nl=PIX // W, w=W)
                for s in range(NCORES):
                    nc.sync.dma_start(
                        vTo[:, s * (PIX // W):(s + 1) * (PIX // W), 0:HD],
                        g1v[s, 2].rearrange("nl w c -> w nl c"))
                E_i = E_s.rearrange("w j ni -> w ni j")
                for sc in range(NCORES):
                    x1blk = p2ds.tile([HD + 1, 32, W], BF16, tag="x1b")
                    for il in range(32):
                        i = sc * 32 + il
                        psX = psp.tile([HD + 1, W], FP32, tag="ps")
                        nc.tensor.matmul(psX[:], vTo[:, i, :], E_i[:, i, :],
                                         start=True, stop=True)
                        drain(i, x1blk[:, il, :], psX[:])
                    nc.sync.dma_start(m2v[sc, 0:33], x1blk[:])
                x2st4 = p2c.tile([128, NI, 32], BF16)
                E_j = E_s
                for g in range(32):
                    ps4 = psp.tile([128, NI], FP32, tag="ps")
                    for a in range(4):
                        j = a * 32 + g
                        nc.tensor.matmul(
                            ps4[a * 32:(a + 1) * 32, :],
                            rvT_t[:, j, :], E_j[:, j, :],
                            start=True, stop=True,
                            tile_position=(0, a * 32))
                    drain(g, x2st4[:, :, g], ps4[:])
                for r in range(NCORES):
                    for a in range(4):
                        nc.sync.dma_start(
                            m2v[r, 33:65, :, a * 32:(a + 1) * 32],
                            x2st4[a * 32:(a + 1) * 32,
                                  r * 32:(r + 1) * 32, :])

            nc.gpsimd.collective_compute(
                "AllToAll", mybir.AluOpType.bypass,
                replica_groups=[list(range(NCORES))],
                ins=[mine2.opt()], outs=[gath2.opt()])

            # =============== phase 3: merge + normalize + wout conv =========
            with tc.tile_pool(name="ph3", bufs=1) as p3, \
                 tc.tile_pool(name="st3", bufs=2) as st3:
                g2v = gath2.rearrange("(hh t f) -> hh t f", hh=NH, t=65)
                y1b = p3.tile([128, 2, PIX], BF16)
                x2b = p3.tile([128, 2, PIX], BF16)
                sums = p3.tile([NH, PIX], BF16)
                for hh in range(NH):
                    kt, po = divmod(hh, 4)
                    nc.sync.dma_start(
                        y1b[po * 32:(po + 1) * 32, kt, :], g2v[hh, 0:HD])
                    nc.sync.dma_start(
                        x2b[po * 32:(po + 1) * 32, kt, :],
                        g2v[hh, 33:33 + HD])
                    nc.sync.dma_start(
                        sums[hh:hh + 1, :], g2v[hh, HD:HD + 1])
                nc.vector.tensor_add(y1b[:], y1b[:], x2b[:])
                gv1_8 = p3.tile([NH, 1], FP32)
                nc.gpsimd.dma_start(
                    gv1_8[:], gv1a.rearrange("(h o) -> h o", o=1))
                nc.scalar.activation(gv1_8[:], gv1_8[:], Tanh)
                rsr = p3.tile([NH, PIX], FP32)
                nc.vector.reciprocal(rsr[:], sums[:])
                nc.vector.tensor_scalar_mul(rsr[:], rsr[:], gv1_8[:])
                rsd = dram.tile([NH, PIX], FP32)
                nc.sync.dma_start(rsd[:], rsr[:])
                rsx = p3.tile([128, 2, PIX], FP32)
                for hh in range(NH):
                    kt, po = divmod(hh, 4)
                    nc.sync.dma_start(
                        rsx[po * 32:(po + 1) * 32, kt, :],
                        rsd[hh:hh + 1, :].to_broadcast((32, PIX)))
                yb = p3.tile([128, 2, PIX], BF16)
                nc.vector.tensor_mul(yb[:], y1b[:], rsx[:])

                wof = p3.tile([128, 2, C], FP32)
                nc.gpsimd.dma_start(
                    wof[:], woutT.rearrange("(kt p) f -> p kt f", p=128))
                wob = p3.tile([128, 2, C], BF16)
                nc.vector.tensor_copy(wob[:], wof[:])
                if DEBUG:
                    with tc.tile_pool(name="dbg", bufs=1) as dbp:
                        ovf = out.rearrange("(pix) c -> (pix c)")
                        def dump(dst_off, src_ap, shape):
                            P0, F0 = shape
                            for f0 in range(0, F0, 1024):
                                fl = min(1024, F0 - f0)
                                t_b = dbp.tile([P0, fl], BF16, tag="db")
                                nc.sync.dma_start(t_b[:], src_ap[:, f0:f0 + fl])
                                t_f = dbp.tile([P0, fl], FP32, tag="df")
                                nc.vector.tensor_copy(t_f[:], t_b[:])
                                nc.sync.dma_start(
                                    ovf[dst_off + f0 * P0:
                                        dst_off + (f0 + fl) * P0].rearrange(
                                        "(p f) -> p f", p=P0), t_f[:])
                        g1f = gath1.rearrange("(a) -> a")
                        dump(0, g1f[0:HD * PIX].rearrange("(c f) -> c f", c=HD),
                             (HD, PIX))                      # q of my head, rank0 pixels
                        g2f = gath2.rearrange("(a) -> a")
                        dump(HD * PIX, g2f[0:33 * PIX].rearrange(
                            "(t f) -> t f", t=33), (33, PIX))  # head0 x1+sums, my pixels
                        e0 = dbp.tile([W, W], FP32, tag="e0")
                        nc.vector.tensor_copy(e0[:], E_s[:, :, 0].rearrange('w j -> w j'))
                        nc.sync.dma_start(
                            ovf[300000:300000 + W * W].rearrange(
                                "(p f) -> p f", p=W), e0[:])   # E_s[:, 0, :]
                        dump(400000, g1f[HD * PIX:2 * HD * PIX].rearrange(
                            "(c f) -> c f", c=HD), (HD, PIX))   # k head, rank0
                if not DEBUG:
                    ov = out.rearrange("(pt p) c -> p pt c", p=128)
                    for pt in range(32):
                        psO = psp.tile([128, C], FP32, tag="ps")
                        for kt in range(2):
                            nc.tensor.matmul(
                                psO[:], yb[:, kt, pt * 128:(pt + 1) * 128],
                                wob[:, kt, :], start=(kt == 0), stop=(kt == 1))
                        so = st3.tile([128, C], FP32, tag="os")
                        drain(pt, so[:], psO[:])
                        nc.sync.dma_start(ov[:, pt, :], so[:])

    nc.finalize()
    return nc


_BUILT = None


def kernel(x, wq, wk, wv, wout, rq, rk, rv, Gq, Gk, Gv1, Gv2):
    global _BUILT
    x = np.asarray(x, np.float32)
    wqkvT = np.concatenate(
        [np.asarray(wq, np.float32).T, np.asarray(wk, np.float32).T,
         np.asarray(wv, np.float32).T], axis=1).copy()
    woutT = np.ascontiguousarray(np.asarray(wout, np.float32).T)
    rq = np.asarray(rq, np.float32)
    rk = np.asarray(rk, np.float32)
    rv = np.asarray(rv, np.float32)
    Gq = np.asarray(Gq, np.float32)
    Gk = np.asarray(Gk, np.float32)
    Gv1 = np.asarray(Gv1, np.float32)
    Gv2 = np.asarray(Gv2, np.float32)

    if _BUILT is None:
        _BUILT = _build()
    nc = _BUILT

    in_maps = []
    for r in range(NCORES):
        xs = np.ascontiguousarray(
            x[:, :, r * HSH:(r + 1) * HSH, :].transpose(1, 0, 2, 3)
        ).reshape(C, PIX)
        in_maps.append({
            "x_s": xs,
            "wqkvT": wqkvT,
            "woutT": woutT,
            "rq_h": np.ascontiguousarray(rq[r]),
            "rk_h": np.ascontiguousarray(rk[r]),
            "rv_h": np.ascontiguousarray(rv[r]),
            "g2": np.array([Gq[r], Gk[r], Gv1[r], Gv2[r]], np.float32),
            "gv1a": Gv1.copy(),
            "gv2a": Gv2.copy(),
        })

    res = run_bass_kernel_spmd(nc, in_maps, core_ids=list(range(NCORES)))
    global _LAST_RESULT
    _LAST_RESULT = res
    outf = np.empty((N, C, H, W), np.float32)
    for r in range(NCORES):
        o = res.results[r]["out"].reshape(N, HSH, W, C).transpose(0, 3, 1, 2)
        outf[:, :, r * HSH:(r + 1) * HSH, :] = o
    return outf
